# revision 1
# baseline (speedup 1.0000x reference)
"""MLP (additive/Bahdanau) attention kernel for Trainium2, 8 NeuronCores.

Reference computation (per batch b):
    q = query[b] @ W_q                      # (Lq, U)
    k = key[b]   @ W_k                      # (Lk, U)
    scores[i,j] = sum_u v_w[u] * tanh(q[i,u] + k[j,u])
    attn = softmax(mask(scores, valid_len[b]))
    out[b] = attn @ value[b]                # (Lq, Dv)

Shapes: B=16, Lq=128, Lk=256, Dq=Dk=Dv=512, U=256, fp32.

Strategy: data-parallel over batch across 8 cores (2 batches per core).
valid_len is known at kernel-build time, so the program only computes
tanh over the valid key range (padded to a common per-slot length so one
SPMD program serves all 8 cores).  Per-core validity is enforced with a
data-driven additive mask (-1e6) applied while draining scores from PSUM.

Per-core device pipeline (slot s with padded key length VL):
  PE  : qfT[u,q] / kfT[u,k] projections (host supplies pre-transposed
        query/key so the contraction dim d is already on partitions)
  DVE+GpSimd : X[:, i] = kfT + qfT[:, q]  (per-q tensor_scalar_add)
  ACT : T = tanh(X) in large instructions
  PE  : scores[q, :] = v^T T_q via 32-column stationary tiles
        (VSTAT[h][j][:, i] = v_half if i == j else 0) accumulated in PSUM
  DVE : scores += mask (PSUM -> SBUF)
  ACT : E = exp(scores), accum_out -> row sums
  DVE : recip = 1/denom
  PE  : E^T via transpose-matmul; out = E^T.T @ value (accum over k tiles)
  DVE : out *= recip (per-partition scalar)
"""

import contextlib

import numpy as np

import concourse.bacc as bacc
import concourse.bass as bass
import concourse.tile as tile
from concourse import mybir
from concourse.bass_utils import run_bass_kernel_spmd

F32 = mybir.dt.float32

B, LQ, LK = 16, 128, 256
D, U, DV = 512, 256, 512
N_CORES = 8
NEG = -1e6
CH = 16          # q-chunk size
MW = 32          # PSUM column-group width (stationary tile columns)


def _pad8(n: int) -> int:
    return max(8, (n + 7) // 8 * 8)


# test/profiling hooks (unused by the grading path)
_RUN_KWARGS: dict = {}
LAST_RESULTS = None


def build_program(VL0: int, VL1: int, repeat: int = 0, hoist_dma: bool = False,
                  stages: str = "full", rotate: bool = True):
    """Build the SPMD Bass program for two batch slots with padded key
    lengths VL0, VL1.

    repeat>0 wraps the compute in a hardware loop (timing measurement only).
    hoist_dma moves the per-slot input DMAs outside the loop.
    stages: "full" | "tanh" (stop after tanh) | "adds" (stop after adds) —
    ablation variants for timing attribution only.
    """
    nc = bacc.Bacc(None, target_bir_lowering=False)

    # All inputs ship as SBUF-row-major "blobs": one DMA each, 128
    # max-length descriptors, instead of thousands of sub-KB descriptors.
    # cblob row p = [wq(4x256) | wk(4x256) | vstat(2x32x32) | id(128)]
    # blob_s row p = [qt(4x128) | kt(4xVL) | val(nkt x 512) | mask(VL)]
    WC = 4 * U + 4 * U + 2 * MW * MW + 128
    nkts = [(VL0 + 127) // 128, (VL1 + 127) // 128]
    WS = [4 * LQ + 4 * VL0 + nkts[0] * DV + VL0,
          4 * LQ + 4 * VL1 + nkts[1] * DV + VL1]
    p_cblob = nc.declare_dram_parameter("cblob", [128, WC], F32, isOutput=False)
    p_blob = [
        nc.declare_dram_parameter("blob0", [128, WS[0]], F32, isOutput=False),
        nc.declare_dram_parameter("blob1", [128, WS[1]], F32, isOutput=False),
    ]
    p_out = nc.declare_dram_parameter("out2", [2, LQ, DV], F32, isOutput=True)

    VLs = [VL0, VL1]

    with tile.TileContext(nc) as tc:
        with (
            tc.tile_pool(name="const", bufs=1) as const,
            tc.tile_pool(name="proj", bufs=2) as proj,
            tc.tile_pool(name="big", bufs=2) as big,
            tc.tile_pool(name="sm", bufs=2) as sm,
            tc.tile_pool(name="ps_proj", bufs=1, space="PSUM") as ps_proj,
            tc.tile_pool(name="ps_pet", bufs=2, space="PSUM") as ps_pet,
            tc.tile_pool(name="ps_sc", bufs=2, space="PSUM") as ps_sc,
            tc.tile_pool(name="ps_at", bufs=1, space="PSUM") as ps_at,
        ):
            # ---- constants (single-DMA blob) ----
            cb = const.tile([128, WC], F32)
            nc.sync.dma_start(out=cb[:], in_=p_cblob[:])
            o = 0
            wq_sb = cb[:, o:o + 4 * U].rearrange("p (a b) -> p a b", a=4); o += 4 * U
            wk_sb = cb[:, o:o + 4 * U].rearrange("p (a b) -> p a b", a=4); o += 4 * U
            vstat_sb = cb[:, o:o + 2 * MW * MW].rearrange(
                "p (a b c) -> p a b c", a=2, b=MW); o += 2 * MW * MW
            id_sb = cb[:, o:o + 128]; o += 128

            def load_slot(s):
                VL = VLs[s]
                nkt = (VL + 127) // 128
                sb = proj.tile([128, WS[s]], F32, tag=f"blob{s}")
                nc.sync.dma_start(out=sb[:], in_=p_blob[s][:])
                o = 0
                qt_sb = sb[:, o:o + 4 * LQ].rearrange("p (a b) -> p a b", a=4); o += 4 * LQ
                kt_sb = sb[:, o:o + 4 * VL].rearrange("p (a b) -> p a b", a=4); o += 4 * VL
                val_sb = sb[:, o:o + nkt * DV].rearrange("p (a b) -> p a b", a=nkt); o += nkt * DV
                mask_sb = sb[:, o:o + VL]; o += VL
                return qt_sb, kt_sb, val_sb, mask_sb

            hoisted = [load_slot(s) for s in range(2)] if hoist_dma else None

            loop_cm = tc.For_i(0, repeat, 1) if repeat else contextlib.nullcontext()
            with loop_cm:
                for s in range(2):
                    VL = VLs[s]
                    nkt = (VL + 127) // 128

                    if hoisted is not None:
                        qt_sb, kt_sb, val_sb, mask_sb = hoisted[s]
                    else:
                        qt_sb, kt_sb, val_sb, mask_sb = load_slot(s)

                    # ---- projections: qfT [u, q], kfT [u, k] per u-half ----
                    qft_sb = proj.tile([128, 2, LQ], F32, tag=f"qft{s}")
                    kft_sb = proj.tile([128, 2, VL], F32, tag=f"kft{s}")
                    for h in range(2):
                        ps_q = ps_proj.tile([128, LQ], F32, tag="pq")
                        for dt in range(4):
                            nc.tensor.matmul(
                                ps_q[:],
                                wq_sb[:, dt, h * 128:(h + 1) * 128],
                                qt_sb[:, dt, :],
                                start=(dt == 0), stop=(dt == 3),
                            )
                        nc.vector.tensor_copy(qft_sb[:, h, :], ps_q[:])
                        ps_k = ps_proj.tile([128, VL], F32, tag="pk")
                        for dt in range(4):
                            nc.tensor.matmul(
                                ps_k[:],
                                wk_sb[:, dt, h * 128:(h + 1) * 128],
                                kt_sb[:, dt, :],
                                start=(dt == 0), stop=(dt == 3),
                            )
                        nc.vector.tensor_copy(kft_sb[:, h, :], ps_k[:])

                    # ---- main: scores[q, k] = sum_u v_u tanh(qf + kf) ----
                    ps_scores = ps_sc.tile([128, VL], F32, tag="sc")
                    first_mm = [None] * 4  # per-group opener (start=True)
                    opener = None
                    if rotate:
                        # Single opener writing EVERY element (start=True):
                        # has_written is set for the full region no matter how
                        # the hardware scopes the start-clear, and it seeds the
                        # additive validity mask for free.  All score matmuls
                        # are then pure accumulates, pinned after it.
                        opener = nc.tensor.matmul(
                            ps_scores[:, :VL], id_sb, mask_sb[:, :VL],
                            start=True, stop=False, skip_group_check=True,
                        )
                    for qc in range(LQ // CH):
                        for h in range(2):
                            x_t = big.tile([128, CH * VLs[0]], F32, tag="x")
                            t_t = big.tile([128, CH * VLs[0]], F32, tag="t")
                            # X[:, i, :] = kfT + qfT[:, qc*CH+i] as broadcast
                            # tensor_tensor adds (stride-0 dims), split ~2:1
                            # between DVE and GpSimd (GpSimd 2-input is ~2x
                            # slower, so it takes the smaller share)
                            k_ap = kft_sb[:, h, :]
                            for eng, i0, i1 in ((nc.vector, 0, CH - 5),
                                                (nc.gpsimd, CH - 5, CH)):
                                nq = i1 - i0
                                in0 = bass.AP(tensor=k_ap.tensor, offset=k_ap.offset,
                                              ap=[list(k_ap.ap[0]), [0, nq], list(k_ap.ap[1])])
                                q_ap = qft_sb[:, h, qc * CH + i0:qc * CH + i1]
                                in1 = bass.AP(tensor=q_ap.tensor, offset=q_ap.offset,
                                              ap=[list(q_ap.ap[0]), list(q_ap.ap[1]), [0, VL]])
                                xv = x_t[:, i0 * VL:i1 * VL].rearrange(
                                    "p (a b) -> p a b", a=nq)
                                eng.tensor_add(xv, in0, in1)
                            if stages == "adds":
                                continue
                            nc.scalar.activation(
                                t_t[:, :CH * VL], x_t[:, :CH * VL],
                                mybir.ActivationFunctionType.Tanh,
                            )
                            if stages == "tanh":
                                continue
                            for i in range(CH):
                                # rotate PSUM column-groups so consecutive
                                # matmuls execute concurrently: q -> (g, j),
                                # psum row = 32*g + j (descrambled at out DMA).
                                # Without rotation groups are q-sequential
                                # (CoreSim-compatible debug mode).
                                q = qc * CH + i
                                if rotate:
                                    g, j = q % 4, q // 4
                                    last = (h == 1 and q == LQ - 1)
                                else:
                                    g, j = q // MW, q % MW
                                    last = (h == 1 and q % MW == MW - 1)
                                mm = nc.tensor.matmul(
                                    ps_scores[MW * g:MW * (g + 1), :VL],
                                    vstat_sb[:, h, j, :],
                                    t_t[:, i * VL:(i + 1) * VL],
                                    start=(False if rotate else first_mm[g] is None),
                                    stop=last,
                                    tile_position=(0, MW * g),
                                    skip_group_check=True,
                                )
                                if rotate:
                                    tile.add_dep_helper(
                                        mm.ins, opener.ins, sync=False,
                                        reason="mask opener first")
                                elif first_mm[g] is None:
                                    first_mm[g] = mm
                                else:
                                    tile.add_dep_helper(
                                        mm.ins, first_mm[g].ins, sync=False,
                                        reason="group opener first")
                    if stages != "full":
                        continue

                    # ---- softmax (no max-subtraction: |scores| <= sum|v|).
                    # rotate: mask already seeded into PSUM by the opener.
                    e_sb = sm.tile([128, VL], F32, tag=f"e{s}")
                    denom = sm.tile([128, 1], F32, tag="den")
                    if rotate:
                        exp_in = ps_scores[:, :VL]
                    else:
                        sc_sb = sm.tile([128, VL], F32, tag=f"scm{s}")
                        nc.vector.tensor_add(sc_sb[:], ps_scores[:, :VL], mask_sb)
                        exp_in = sc_sb[:]
                    nc.scalar.activation(
                        e_sb[:], exp_in,
                        mybir.ActivationFunctionType.Exp,
                        accum_out=denom[:],
                    )
                    recip = sm.tile([128, 1], F32, tag="rec")
                    nc.vector.reciprocal(recip[:], denom[:])

                    # ---- attn @ value ----
                    ps_out = ps_at.tile([128, DV], F32, tag="po")
                    for kt in range(nkt):
                        w = min(128, VL - kt * 128)
                        ps_et = ps_pet.tile([128, 128], F32, tag="pet")
                        nc.tensor.transpose(ps_et[:w, :], e_sb[:, kt * 128:kt * 128 + w], id_sb)
                        et_sb = sm.tile([128, 128], F32, tag="et")
                        nc.vector.tensor_copy(et_sb[:w, :], ps_et[:w, :])
                        nc.tensor.matmul(
                            ps_out[:],
                            et_sb[:w, :],
                            val_sb[:w, kt, :],
                            start=(kt == 0), stop=(kt == nkt - 1),
                        )
                    out_sb = sm.tile([128, DV], F32, tag="out")
                    nc.vector.tensor_scalar_mul(out_sb[:], ps_out[:], recip[:])
                    if rotate:
                        # rows are scrambled q -> 32*(q%4) + q//4; invert via
                        # per-group DMAs with a row-strided DRAM pattern:
                        # group g holds q = 4*j + g for j in [0, 32)
                        po = p_out[s]
                        for g in range(4):
                            dst = bass.AP(
                                tensor=po.tensor,
                                offset=po.offset + g * DV,
                                ap=[[4 * DV, MW], [1, DV]],
                            )
                            nc.sync.dma_start(
                                out=dst, in_=out_sb[MW * g:MW * (g + 1), :])
                    else:
                        nc.sync.dma_start(out=p_out[s], in_=out_sb[:])

    nc.finalize()
    return nc


def prepare(query, key, value, valid_len, W_q, W_k, v_w):
    query = np.ascontiguousarray(np.asarray(query, dtype=np.float32))
    key = np.ascontiguousarray(np.asarray(key, dtype=np.float32))
    value = np.ascontiguousarray(np.asarray(value, dtype=np.float32))
    W_q = np.ascontiguousarray(np.asarray(W_q, dtype=np.float32))
    W_k = np.ascontiguousarray(np.asarray(W_k, dtype=np.float32))
    v_w = np.ascontiguousarray(np.asarray(v_w, dtype=np.float32))
    vl = np.asarray(valid_len).astype(np.int64)

    # ---- batch -> (core, slot) assignment: slot0 = top-8 by valid_len ----
    order = np.argsort(-vl, kind="stable")
    slot0 = [int(order[c]) for c in range(N_CORES)]
    slot1 = [int(order[N_CORES + c]) for c in range(N_CORES)]
    VL0 = _pad8(int(vl[slot0[0]]))
    VL1 = _pad8(int(vl[slot1[0]]))

    # ---- host-side constant blob ----
    vstat = np.zeros((128, 2, MW, MW), np.float32)
    for h in range(2):
        for j in range(MW):
            vstat[:, h, j, j] = v_w[h * 128:(h + 1) * 128]
    id128 = np.eye(128, dtype=np.float32)
    # cblob row p = [wq(4,U) | wk(4,U) | vstat(2,MW,MW) | id(128)]
    wq_t = W_q.reshape(4, 128, U).transpose(1, 0, 2).reshape(128, 4 * U)
    wk_t = W_k.reshape(4, 128, U).transpose(1, 0, 2).reshape(128, 4 * U)
    cblob = np.ascontiguousarray(np.concatenate(
        [wq_t, wk_t, vstat.reshape(128, -1), id128], axis=1))

    def mk_blob(bi, VL):
        nkt = (VL + 127) // 128
        qt = query[bi].T.reshape(4, 128, LQ).transpose(1, 0, 2).reshape(128, 4 * LQ)
        kt = key[bi].T[:, :VL].reshape(4, 128, VL).transpose(1, 0, 2).reshape(128, 4 * VL)
        val = np.zeros((128, nkt, DV), np.float32)
        for t in range(nkt):
            w = min(128, VL - t * 128)
            val[:w, t, :] = value[bi][t * 128:t * 128 + w]
        mrow = np.where(np.arange(VL)[None, :] < vl[bi], 0.0, NEG).astype(np.float32)
        mask = np.broadcast_to(mrow, (128, VL))
        return np.ascontiguousarray(np.concatenate(
            [qt, kt, val.reshape(128, -1), mask], axis=1).astype(np.float32))

    in_maps = []
    for c in range(N_CORES):
        b0, b1 = slot0[c], slot1[c]
        m = {
            "cblob": cblob,
            "blob0": mk_blob(b0, VL0),
            "blob1": mk_blob(b1, VL1),
        }
        in_maps.append(m)

    return in_maps, slot0, slot1, VL0, VL1, vl, value


ROTATE = True


def kernel(query, key, value, valid_len, W_q, W_k, v_w):
    in_maps, slot0, slot1, VL0, VL1, vl, value = prepare(
        query, key, value, valid_len, W_q, W_k, v_w)

    nc = build_program(VL0, VL1, rotate=ROTATE)
    res = run_bass_kernel_spmd(nc, in_maps, list(range(N_CORES)), **_RUN_KWARGS)
    global LAST_RESULTS
    LAST_RESULTS = res

    out = np.empty((B, LQ, DV), np.float32)
    for c in range(N_CORES):
        out[slot0[c]] = res.results[c]["out2"][0]
        out[slot1[c]] = res.results[c]["out2"][1]

    # degenerate batches (valid_len == 0): reference softmax over all-masked
    # scores is uniform over all LK keys -> output = mean of value rows
    for b in range(B):
        if vl[b] == 0:
            out[b] = value[b].mean(axis=0, keepdims=True)
    return out

